# revision 23
# baseline (speedup 1.0000x reference)
"""Trainium2 Bass kernel for nn_MaxCDFdp_multiclass.

Computes max over (class, probe) of |ECDF0 - ECDF1| where the ECDFs are
sigmoid-smoothed empirical CDFs of y_pred per class, for the two groups
defined by s in {0,1}.

v4: narrow-window evaluation with fp16 operands, ACT-bound pipeline.
sigmoid(10*(grid - y)) is within ~7e-3 of {0,1} outside |grid - y| <=
MARGIN=0.5, and the resulting one-sided per-cell bias (~0.04*sigma(-5))
cancels between the two groups, so only W=16 of the 100 probes need
evaluation per (sample, class) tile. Host sorts each group per class,
cuts sorted samples into tiles of <=128 rows whose per-class span fits
the W-probe window, and picks a per-(tile, class) window base B:
probes >= B+W-1 are treated as saturated (the window's last column is
added to all later probes on host); probes < B are dropped.

Device per group of G=6 tiles (all operands fp16 for DVE 2x mode; the
diff layout is [w-outer, c-inner] so every tensor_tensor operand has a
packed 2-byte innermost dim):
  DVE: diff[128, g, W, C] = Dj_bcast + A_bcast     (one op, 0.52ns/col)
  ACT: sig = sigmoid(10*diff) -> fp16              (the pacing engine)
  PE : acc[chunk, 2] += sig_chunk.T @ ind[128, 2]  (3 chunks/tile, PSUM)
Final: DVE drains the single-bank PSUM acc to SBUF, one DMA out.
Host: relocate each tile's [2, W, C] window into [2, C, P] at its B
offsets (+ saturated tail), sum over cores, divide by group counts,
abs, max.

Validated ~2e-3 relative against the exact reference in numpy
simulation (tolerance 2e-2); error is dominated by fp16/sigmoid-LUT
quantization noise which averages out across the ~25k samples.
"""

import os
from contextlib import ExitStack

import numpy as np

import concourse.bass as bass
import concourse.bacc as bacc
import concourse.tile as tile
from concourse import mybir
from concourse.bass_utils import run_bass_kernel_spmd

N, C, P = 50000, 20, 100
TEMP = 10.0
NCORES = 8
PART = 128
W = 16                 # probe-window width per tile
CW = C * W             # 320
G = 8                  # max tiles per device group
MARGIN = 0.45          # |grid - y| cutoff (4.5 in sigmoid-arg units)

_F32 = mybir.dt.float32
_F16 = mybir.dt.float16

# the [<=128-col] chunks of CW that become matmul stationary operands
_QCH = [(q * 128, min((q + 1) * 128, CW)) for q in range(-(-CW // 128))]
_NQ = len(_QCH)     # 3
_SLOT = _NQ         # 3 psum cols per tile (1-col indicator)

_CACHED = {}


def _groups(T):
    # small first group (ACT can start right after its table load) and
    # small-ish last group (short PE drain tail)
    # ramp up so the pipeline fills while the input DMAs land, big G-tile
    # groups in the middle to amortize per-instruction overhead, taper at
    # the end so the PE/drain tail after the last sigmoid stays short
    groups = []
    i = 0
    n = 1
    while n < G:
        if i >= T:
            return groups
        n = min(n, T - i)
        groups.append((i, n))
        i += n
        n *= 2
    while T - i > G + 4:
        groups.append((i, G))
        i += G
    rem = T - i
    if rem > 4:
        groups.append((i, rem - 4))
        i += rem - 4
    for n in (2, 2):
        n = min(n, T - i)
        if n:
            groups.append((i, n))
            i += n
    return groups


def _split_tile(T, groups):
    # input DMA A-spans: b1 = first group only (smallest possible first
    # transfer), b2 = next few groups, b3 = the rest; t_d = accumulator
    # split before the last two groups for an early drain
    n1 = min(3, len(groups))
    t1 = groups[n1 - 1][0] + groups[n1 - 1][1]
    n2 = min(n1 + 2, len(groups))
    t2 = groups[n2 - 1][0] + groups[n2 - 1][1]
    n3 = max(0, len(groups) - 3)
    t_d = groups[n3][0] if n3 < len(groups) else T
    return t1, t2, t_d


def _build_bass(T):
    # blob free-dim layout: [Dj: W*C][ind: T*2][A: T*C], all fp16.
    # Exactly two input DMAs (extra dma_starts stall ~4.5us each on the
    # DGE credit ring): DMA1 = dj+ind+A[:t_a] (everything the first three
    # groups touch), DMA2 = the remaining A.
    dw, iw, aw = CW, T, T * C
    blob_w = dw + iw + aw
    ow = _SLOT * T
    nc = bacc.Bacc(None, target_bir_lowering=False)
    b_d = nc.dram_tensor("b", [PART, blob_w], _F16, kind="ExternalInput")
    o_d = nc.dram_tensor("o", [PART, ow], _F32, kind="ExternalOutput")

    groups = _groups(T)
    t1, t2, t_d = _split_tile(T, groups)

    with ExitStack() as ctx:
        tc = ctx.enter_context(tile.TileContext(nc))
        constp = ctx.enter_context(tc.tile_pool(name="const", bufs=1))
        diffp = ctx.enter_context(tc.tile_pool(name="diff", bufs=3))
        sigp = ctx.enter_context(tc.tile_pool(name="sig", bufs=4))
        psump = ctx.enter_context(
            tc.tile_pool(name="psum", bufs=1, space=bass.MemorySpace.PSUM)
        )
        outp = ctx.enter_context(tc.tile_pool(name="outp", bufs=1))

        s1 = dw + iw + t1 * C
        s2 = dw + iw + t2 * C
        b1 = constp.tile([PART, s1], _F16)
        nc.sync.dma_start(b1[:], b_d[:, 0:s1])
        b2 = constp.tile([PART, s2 - s1], _F16)
        nc.sync.dma_start(b2[:], b_d[:, s1:s2])
        b3 = constp.tile([PART, blob_w - s2], _F16)
        nc.sync.dma_start(b3[:], b_d[:, s2:])
        dj_sb = b1[:, 0:dw].rearrange("p (w c) -> p w c", w=W)
        ind_sb = b1[:, dw : dw + iw].rearrange("p (t g) -> p t g", t=T)  # g=1
        a_srcs = [
            (0, t1, b1[:, dw + iw :].rearrange("p (t c) -> p t c", c=C)),
            (t1, t2, b2[:].rearrange("p (t c) -> p t c", c=C)),
            (t2, T, b3[:].rearrange("p (t c) -> p t c", c=C)),
        ]

        # all tiles' reductions land here: tile i, chunk q, group g at
        # column i*_SLOT + 2q + g; rows = wc-position within the chunk.
        # acc1 (tiles < t_d) drains early, under the last groups' compute.
        acc1 = psump.tile([PART, _SLOT * t_d], _F32)
        acc2 = psump.tile([PART, ow - _SLOT * t_d], _F32)

        def acc_slice(i, q):
            col = i * _SLOT + q
            if i < t_d:
                return acc1[:, col : col + 1]
            col -= _SLOT * t_d
            return acc2[:, col : col + 1]

        out1 = outp.tile([PART, _SLOT * t_d], _F32)
        out2 = outp.tile([PART, ow - _SLOT * t_d], _F32)

        for g0, gn in groups:
            diff = diffp.tile([PART, G, W, C], _F16, tag="diff")
            dj_v = dj_sb.unsqueeze(1).broadcast_to([PART, gn, W, C])
            a_sb, off = next(
                (src, lo) for lo, hi, src in a_srcs if lo <= g0 < hi
            )
            a_v = (
                a_sb[:, g0 - off : g0 - off + gn, :]
                .unsqueeze(2)
                .broadcast_to([PART, gn, W, C])
            )
            nc.vector.tensor_add(diff[:, 0:gn], dj_v, a_v)

            sig = sigp.tile([PART, G, W, C], _F16, tag="sig")
            nc.scalar.activation(
                sig[:, 0:gn], diff[:, 0:gn],
                mybir.ActivationFunctionType.Sigmoid, scale=TEMP,
            )
            sig_f = sig[:].rearrange("p t w c -> p t (w c)")

            for t in range(gn):
                i = g0 + t
                for q, (c0, c1) in enumerate(_QCH):
                    nc.tensor.matmul(
                        acc_slice(i, q)[0 : c1 - c0, :],
                        sig_f[:, t, c0:c1],
                        ind_sb[:, i, :],
                        start=True,
                        stop=True,
                    )

        # drains sit after all ADDs in the in-order Vector queue; the big
        # acc1 copy only waits on PE through tile t_d, so it overlaps the
        # last two groups' sigmoids instead of serializing after them
        nc.vector.tensor_copy(out1[:], acc1[:])
        nc.sync.dma_start(o_d[:, 0 : _SLOT * t_d], out1[:])
        nc.vector.tensor_copy(out2[:], acc2[:])
        nc.sync.dma_start(o_d[:, _SLOT * t_d :], out2[:])

    nc.finalize()
    return nc


def _get_nc(T):
    if T not in _CACHED:
        _CACHED[T] = _build_bass(T)
    return _CACHED[T]


# test.py reads this after calling kernel() for profiling info
LAST_RESULTS = None
LAST_DELTA = None


def kernel(y_pred: np.ndarray, s: np.ndarray) -> np.ndarray:
    global LAST_RESULTS
    y = np.ascontiguousarray(np.asarray(y_pred), dtype=np.float32)
    s_np = np.asarray(s)
    assert y.shape == (N, C)

    mn = y.min(axis=0)
    mx = y.max(axis=0)
    step = (mx.astype(np.float64) - mn) / (P - 1)  # f64 for window math

    srt0 = np.sort(y[s_np == 0], axis=0)  # [n0, C], per-class sorted
    srt1 = np.sort(y[s_np == 1], axis=0)
    n0, n1 = srt0.shape[0], srt1.shape[0]

    smax = (W - 2) * step - 2 * MARGIN

    # global tile list over both groups: (gi, vals[cnt, C])
    tiles = []
    for gi, blk in enumerate((srt0, srt1)):
        m = blk.shape[0]
        start = 0
        while start < m:
            end = min(start + PART, m)
            lim = m
            for c in range(C):
                e = np.searchsorted(blk[:, c], blk[start, c] + smax[c], "right")
                lim = min(lim, e)
            end = min(end, max(lim, start + 1))
            tiles.append((gi, blk[start:end]))
            start = end

    # deal round-robin across cores so cut tiles spread evenly
    core_tiles = [tiles[r::NCORES] for r in range(NCORES)]
    T = max(len(t) for t in core_tiles)

    jj = np.arange(W, dtype=np.float32)
    dj = (step.astype(np.float32)[:, None] * jj[None, :]).astype(np.float32)
    dj_wc = np.ascontiguousarray(dj.T)  # [W, C]

    dw, iw, aw = CW, T, T * C
    in_maps = []
    b_tabs = []
    for r in range(NCORES):
        ctiles = core_tiles[r]
        A = np.zeros((PART, T, C), np.float16)
        ind = np.zeros((PART, T, 1), np.float16)
        Btab = np.zeros((T, C), np.int32)
        for t, (gi, vals) in enumerate(ctiles):
            cnt = vals.shape[0]
            ymax_t = vals.max(axis=0).astype(np.float64)
            B = np.ceil((ymax_t + MARGIN - mn) / step).astype(np.int64) - W + 1
            B = np.clip(B, 0, P - W)
            Btab[t] = B
            base = (mn + step * B).astype(np.float32)  # [C]
            A[:cnt, t, :] = (base[None, :] - vals).astype(np.float16)
            A[cnt:, t, :] = (base[None, :] - vals[-1]).astype(np.float16)
            ind[:cnt, t, 0] = 1.0  # tile is single-group; host adds into gi
        blob = np.empty((PART, blob_w := dw + iw + aw), np.float16)
        blob[:, 0:dw] = np.broadcast_to(dj_wc.reshape(1, dw), (PART, dw))
        blob[:, dw : dw + iw] = ind.reshape(PART, iw)
        blob[:, dw + iw :] = A.reshape(PART, aw)
        in_maps.append({"b": blob})
        b_tabs.append(Btab)

    nc = _get_nc(T)
    res = run_bass_kernel_spmd(
        nc,
        in_maps,
        core_ids=list(range(NCORES)),
        trace=bool(int(os.environ.get("BASS_KERNEL_TRACE", "0"))),
    )
    LAST_RESULTS = res

    full = np.zeros((2, C, P + W), np.float32)  # halo simplifies the tail add
    for r in range(NCORES):
        o = res.results[r]["o"]  # [128, _SLOT*T]
        # reassemble to [T, W, C]
        arr = np.empty((CW, T), np.float32)
        ot = o.reshape(PART, T, _SLOT)
        for q, (c0, c1) in enumerate(_QCH):
            arr[c0:c1] = ot[0 : c1 - c0, :, q]
        arr = arr.reshape(W, C, T).transpose(2, 0, 1)  # [T, W, C]
        Btab = b_tabs[r]
        for t, (gi, _) in enumerate(core_tiles[r]):
            for c in range(C):
                B = Btab[t, c]
                full[gi, c, B : B + W] += arr[t, :, c]
                full[gi, c, B + W :] += arr[t, W - 1, c]
    full = full[:, :, :P]
    delta = np.abs(full[0] / np.float32(n0) - full[1] / np.float32(n1))
    global LAST_DELTA
    LAST_DELTA = delta
    return np.array(delta.max(), dtype=np.float32)
